# revision 25
# baseline (speedup 1.0000x reference)
"""Single-head attention (B=8, T=2048, E=1024, D=128) on 8 Trainium2 NeuronCores.

Strategy (data-parallel over batch, one batch element per core):
  host: pre-transpose x -> xT[b] = x[b].T (E on rows), cast to fp16;
        pre-scale q/k biases by D**-0.25.
  device, per core (flash-style, pipelined over T-quarters):
    - x streams in [128,1024] chunks on the two HW DGE queues (sync +
      scalar engines), weights interleaved so each lands just before
      first use; only the first ~8 queue slots execute eagerly, the rest
      are paced by the tile clock, so the order here matters a lot
    - 12 junk warm-up matmuls on the first weight chunk keep the PE busy
      through the DMA lead-in so the HAM clock-gate opens (2.4 GHz)
      before real work -- worth ~24us(!) on a ~90us kernel
    - per quarter h: kT/qT (fp16, scaled by D**-0.25) and vT via PE
      (fp16 matmuls, full rate); V natural [k, D] bf16 via PE transposes
    - score/AV "units" interleave with projections as soon as their kT
      block + qT span exist (sweep A: spans 0,1; sweep B: spans 2,3 --
      so 2 persistent ot banks + 4 score banks + 2 proj banks = 8):
        unit(kb-pair, s): 2 score MMs into a 2-bank PSUM pair,
        ONE exp over [128,1024] PSUM->SBUF bf16 (halving ACT overhead),
        2 AV MMs accumulating OT[d, q] into the span's persistent bank,
        softmax denominator accumulated on DVE in bf16 (2x mode; this
        replaces v1's 64 ones-matmuls, ~17us of PE time)
    - span finalize: l = ones[128,1].T @ acc (M=1 matmul); 1/l =
      exp(-ln(l)) on ACT (this walrus rejects custom-DVE reciprocal and
      plain DVE reciprocal is ~8 cyc/elem); broadcast across partitions
      via a K=1 matmul; OT * (1/l) on DVE; DMA out via sync queue only
      (the scalar queue shares the ACT engine, which exp saturates)
    - store outT [D, T] f32; host transposes back to [T, D].
"""

import os
import sys

for _p in ("/opt/trn_rl_repo",):
    if _p not in sys.path and os.path.isdir(_p):
        sys.path.append(_p)

import numpy as np

import concourse.bass as bass
import concourse.tile as tile
from concourse import mybir
from concourse.masks import make_identity
from concourse.vector_clock import ScopedClock

B, T, E, D = 8, 2048, 1024, 128
EC = E // 128           # 8 E-chunks of 128 partitions
NSPAN = 4               # query spans of 512
SPAN = T // NSPAN       # 512
NKB = T // 128          # 16 key blocks
NPAIR = NKB // 2        # 8 key-block pairs
F32 = mybir.dt.float32
F32R = mybir.dt.float32r
BF16 = mybir.dt.bfloat16
F16 = mybir.dt.float16

_MAX_DRAIN_WAITS = 1


def _drain_and_barrier_split(self, tick_clock, wait_clock):
    # This walrus build rejects CTRL instructions carrying more than one sync
    # wait, so spread the kernel-tail drain's waits over single-wait NOPs.
    nc = self.nc
    collector = nc.sync.nop(nofuse=True, hint="drain_wait_collector")
    wait_clock.add_sem_waits(
        collector.ins, ScopedClock({None: tick_clock.global_clock})
    )
    si = collector.ins.sync_info
    waits = list(si.on_wait) if si and si.on_wait else []
    if len(waits) > _MAX_DRAIN_WAITS:
        si.on_wait = waits[:_MAX_DRAIN_WAITS]
        rest = waits[_MAX_DRAIN_WAITS:]
        while rest:
            chunk, rest = rest[:_MAX_DRAIN_WAITS], rest[_MAX_DRAIN_WAITS:]
            extra = nc.sync.nop(nofuse=True, hint="drain_wait_extra")
            if extra.ins.sync_info is None:
                extra.ins.sync_info = type(si)(on_wait=chunk, on_update=[])
            else:
                extra.ins.sync_info.on_wait = chunk

    nc.sync.drain()

    nc.all_engine_barrier()
    assert self.sems is not None
    popped = nc._tile_sem_poison_stack.pop()
    assert popped is self._sem_poison
    nc.clear_and_free_semaphores(list(self.sems.allocated().values()))
    nc.all_engine_barrier()


tile.TileContext._drain_and_barrier = _drain_and_barrier_split


def _split_excess_waits(nc):
    """Walrus in this env allows at most one sync wait per instruction;
    hoist extra waits onto same-engine NOPs placed just before."""
    import copy

    m = nc.m
    cnt = 0
    new_funcs = []
    for function in m.functions:
        new_function = copy.replace(function, blocks=[])
        new_function.set_allocations_from_list(function.allocations)
        for block in function.blocks:
            new_insts = []
            for inst in block.instructions:
                si = inst.sync_info
                waits = list(si.on_wait) if si and si.on_wait else []
                if len(waits) > 1:
                    for w in waits[:-1]:
                        nop = mybir.InstNoOp(name=f"I-swsplit-{cnt}",
                                             ins=[], outs=[])
                        cnt += 1
                        nop.engine = inst.engine
                        nop.sync_info = mybir.SyncInfo(on_wait=[w],
                                                       on_update=[])
                        new_insts.append(nop)
                    si.on_wait = [waits[-1]]
                new_insts.append(inst)
            new_function.blocks.append(
                copy.replace(block, instructions=new_insts))
        new_funcs.append(new_function)
    new_m = copy.replace(m, functions=[])
    for f in new_funcs:
        new_m.functions.append(f)
    nc.m = new_m
    return cnt


def build_nc():
    SCALE = float(np.float32(D) ** np.float32(-0.25))
    H4 = T // 4             # 512, quarter width
    A = mybir.AluOpType

    nc = bass.Bass()
    xT = nc.declare_dram_parameter("xT", [E, T], F16, isOutput=False)[:]
    Wq = nc.declare_dram_parameter("Wq", [128, EC * D], F16, isOutput=False)[:]
    Wk = nc.declare_dram_parameter("Wk", [128, EC * D], F16, isOutput=False)[:]
    Wv = nc.declare_dram_parameter("Wv", [128, EC * D], F16, isOutput=False)[:]
    bqc = nc.declare_dram_parameter("bqc", [D], F32, isOutput=False)[:]
    bkc = nc.declare_dram_parameter("bkc", [D], F32, isOutput=False)[:]
    bvc = nc.declare_dram_parameter("bvc", [D], F32, isOutput=False)[:]
    ident_d = nc.declare_dram_parameter("ident", [128, 128], F32,
                                        isOutput=False)[:]
    onec_d = nc.declare_dram_parameter("onec", [128], BF16, isOutput=False)[:]
    oner_d = nc.declare_dram_parameter("oner", [128], F16, isOutput=False)[:]
    outT = nc.declare_dram_parameter("outT", [D, T], F32, isOutput=True)[:]

    with tile.TileContext(nc) as tc, \
         tc.tile_pool(name="consts", bufs=1) as consts, \
         tc.tile_pool(name="xpool", bufs=1) as xpool, \
         tc.tile_pool(name="persist", bufs=1) as persist, \
         tc.tile_pool(name="vtq", bufs=2) as vtqp, \
         tc.tile_pool(name="ppool", bufs=6) as ppool, \
         tc.tile_pool(name="fin", bufs=2) as finp, \
         tc.tile_pool(name="psO", bufs=2, space="PSUM") as psO, \
         tc.tile_pool(name="psS", bufs=2, space="PSUM") as psS, \
         tc.tile_pool(name="psP", bufs=2, space="PSUM") as psP:

        # ---- constants / weights (spread across both DGE queues) ----
        wq_s = consts.tile([128, EC, D], F16, tag="wq")
        wk_s = consts.tile([128, EC, D], F16, tag="wk")
        wv_s = consts.tile([128, EC, D], F16, tag="wv")
        wk_r = Wk.rearrange("p (c d) -> p c d", d=D)
        bq_s = consts.tile([128, 1], F32, tag="bq")
        bk_s = consts.tile([128, 1], F32, tag="bk")
        bv_s = consts.tile([128, 1], F32, tag="bv")
        ident = consts.tile([128, 128], F32, tag="ident")
        onec = consts.tile([128, 1], BF16, tag="onec")
        oner = consts.tile([1, 128], F16, tag="oner")

        # ---- x: [128,1024] chunks (half-T per E-chunk), 2 HW DGE queues;
        # weights interleaved so each tensor lands just before first use ----
        xh = [[xpool.tile([128, 2 * H4], F16, tag=f"x{e}_{half}",
                          name=f"xh{e}_{half}")
               for half in range(2)] for e in range(EC)]

        def xdma(eng, e, half):
            eng.dma_start(out=xh[e][half],
                          in_=xT[e * 128:(e + 1) * 128,
                                 half * 2 * H4:(half + 1) * 2 * H4])

        # sync queue: wk chunks first so k-proj starts immediately
        nc.sync.dma_start(out=wk_s[:, 0:2, :], in_=wk_r[:, 0:2, :])
        xdma(nc.sync, 0, 0)
        xdma(nc.sync, 2, 0)
        nc.sync.dma_start(out=wk_s[:, 2:EC, :], in_=wk_r[:, 2:EC, :])
        xdma(nc.sync, 4, 0)
        xdma(nc.sync, 6, 0)
        for e in (0, 2, 4, 6):
            xdma(nc.sync, e, 1)
        # scalar queue: x odds + weights interleaved; tiny consts after
        # (each DMA costs ~0.6us of queue time regardless of size, so
        # front-loading 6 tiny consts would starve the k projection)
        xdma(nc.scalar, 1, 0)
        xdma(nc.scalar, 3, 0)
        nc.scalar.dma_start(out=wv_s, in_=Wv.rearrange("p (c d) -> p c d",
                                                       d=D))
        xdma(nc.scalar, 5, 0)
        xdma(nc.scalar, 7, 0)
        nc.scalar.dma_start(out=wq_s,
                            in_=Wq.rearrange("p (c d) -> p c d", d=D))
        for b_s, b_d in ((bq_s, bqc), (bk_s, bkc), (bv_s, bvc)):
            nc.scalar.dma_start(out=b_s, in_=b_d.unsqueeze(1))
        nc.scalar.dma_start(out=ident, in_=ident_d)
        for e in (1, 3, 5, 7):
            xdma(nc.scalar, e, 1)
        nc.scalar.dma_start(out=onec, in_=onec_d.unsqueeze(1))
        nc.scalar.dma_start(out=oner, in_=oner_d.unsqueeze(0))

        # ---- HAM warm-up: junk matmuls on the first-arrived weight chunk
        # fill the DMA lead-in so the PE clock gate opens before real work
        n_dummy = int(os.environ.get("DUMMY_MM", "12"))
        if n_dummy:
            dmy_ps = psP.tile([128, 256], F32, tag="pj", name="dummy_ps")
            wk01 = wk_s.rearrange("p c d -> p (c d)")[:, 0:2 * D]
            for _ in range(n_dummy):
                nc.tensor.matmul(dmy_ps, wk_s[:, 0, :], wk01,
                                 start=True, stop=True)

        def xq(e, h):
            # [128, 512] view of quarter h of E-chunk e
            return xh[e][h // 2][:, (h % 2) * H4:(h % 2 + 1) * H4]

        kT_s = persist.tile([128, T], F16, tag="kT")
        qT_s = persist.tile([128, T], F16, tag="qT")
        V_s = persist.tile([128, NKB, D], BF16, tag="V")
        acc = [persist.tile([128, SPAN], BF16, tag=f"acc{s}", name=f"acc{s}")
               for s in range(NSPAN)]

        ot_ps = [None] * NSPAN      # span -> persistent PSUM bank
        done_in_span = [0] * NSPAN  # kb-pairs accumulated so far

        def evict_scaled(dst, src, bias, on_act):
            # dst = src * SCALE + bias (bias pre-scaled on host)
            if on_act:
                nc.scalar.activation(out=dst, in_=src, scale=SCALE,
                                     bias=bias,
                                     func=mybir.ActivationFunctionType
                                     .Identity)
            else:
                nc.vector.tensor_scalar(out=dst, in0=src,
                                        scalar1=SCALE, scalar2=bias,
                                        op0=A.mult, op1=A.add)

        def qproj(h):
            hsl = slice(h * H4, (h + 1) * H4)
            q_ps = psP.tile([128, H4], F32, tag="pj", name=f"q_ps{h}")
            for e in range(EC):
                nc.tensor.matmul(q_ps, wq_s[:, e, :], xq(e, h),
                                 start=(e == 0), stop=(e == EC - 1))
            evict_scaled(qT_s[:, hsl], q_ps, bq_s, on_act=False)

        def proj_quarter(h, do_q=True):
            hsl = slice(h * H4, (h + 1) * H4)
            k_ps = psP.tile([128, H4], F32, tag="pj", name=f"k_ps{h}")
            for e in range(EC):
                nc.tensor.matmul(k_ps, wk_s[:, e, :], xq(e, h),
                                 start=(e == 0), stop=(e == EC - 1))
            evict_scaled(kT_s[:, hsl], k_ps, bk_s, on_act=False)
            v_ps = psP.tile([128, H4], F32, tag="pj", name=f"v_ps{h}")
            for e in range(EC):
                nc.tensor.matmul(v_ps, wv_s[:, e, :], xq(e, h),
                                 start=(e == 0), stop=(e == EC - 1))
            vtq = vtqp.tile([128, H4], F32, tag="vtq", name=f"vtq{h}")
            nc.vector.tensor_scalar(out=vtq, in0=v_ps,
                                    scalar1=bv_s, scalar2=None, op0=A.add)
            if do_q:
                qproj(h)
            # V natural layout [k, D] for the 4 new key blocks
            for j in range(4):
                kb = 4 * h + j
                vt_ps = psP.tile([128, 128], F32, tag="pj", name=f"vt{kb}")
                nc.tensor.transpose(vt_ps, vtq[:, j * 128:(j + 1) * 128],
                                    ident)
                nc.vector.tensor_copy(out=V_s[:, kb, :], in_=vt_ps)

        def unit_front(p, s):
            # scores + exp for one kb-pair (blocks 2p, 2p+1) against span s
            ssl = slice(s * SPAN, (s + 1) * SPAN)
            st2 = psS.tile([128, 2 * SPAN], F32, tag="st", name=f"st{p}_{s}")
            for i in range(2):
                kb = 2 * p + i
                nc.tensor.matmul(st2[:, i * SPAN:(i + 1) * SPAN],
                                 kT_s[:, kb * 128:(kb + 1) * 128],
                                 qT_s[:, ssl], start=True, stop=True)
            p2 = ppool.tile([128, 2 * SPAN], BF16, tag="p2", name=f"p{p}_{s}")
            nc.scalar.activation(out=p2, in_=st2,
                                 func=mybir.ActivationFunctionType.Exp)
            return p2

        def unit_back(p, s, p2):
            # AV accumulation + denominator accumulation
            if ot_ps[s] is None:
                ot_ps[s] = psO.tile([128, SPAN], F32, tag="ot",
                                    name=f"ot{s}")
            first = done_in_span[s] == 0
            for i in range(2):
                kb = 2 * p + i
                nc.tensor.matmul(ot_ps[s], V_s[:, kb, :],
                                 p2[:, i * SPAN:(i + 1) * SPAN],
                                 start=(first and i == 0),
                                 stop=(done_in_span[s] == NPAIR - 1
                                       and i == 1))
            # denominator partial sums (bf16, values <= ~2.4e3)
            if first:
                nc.vector.tensor_tensor(out=acc[s], in0=p2[:, 0:SPAN],
                                        in1=p2[:, SPAN:], op=A.add)
            else:
                nc.vector.tensor_tensor(out=acc[s], in0=acc[s],
                                        in1=p2[:, 0:SPAN], op=A.add)
                nc.vector.tensor_tensor(out=acc[s], in0=acc[s],
                                        in1=p2[:, SPAN:], op=A.add)
            done_in_span[s] += 1

        def unit(p, s):
            unit_back(p, s, unit_front(p, s))

        def finalize(s):
            # 1/l = exp(-ln(l)) on ACT -- walrus here rejects the custom-DVE
            # fast-reciprocal, and plain DVE reciprocal is ~8 cyc/elem.
            # Chain is column-half pipelined to shorten the exposed tail.
            lr_ps = psP.tile([1, SPAN], F32, tag="pj", name=f"lr{s}")
            nc.tensor.matmul(lr_ps, onec, acc[s], start=True, stop=True)
            lg = finp.tile([1, SPAN], F32, tag="lg", name=f"lg{s}")
            rl16 = finp.tile([1, SPAN], F16, tag="rl16", name=f"rl16{s}")
            rlb_ps = psP.tile([128, SPAN], F32, tag="pj", name=f"rlb{s}")
            rlb = finp.tile([128, SPAN], F32, tag="rlb", name=f"rlbs{s}")
            outsp = finp.tile([128, SPAN], F32, tag="out", name=f"out{s}")
            half = SPAN // 2
            for i in range(2):
                hs = slice(i * half, (i + 1) * half)
                nc.scalar.activation(out=lg[:, hs], in_=lr_ps[:, hs],
                                     func=mybir.ActivationFunctionType.Ln)
                nc.scalar.activation(out=rl16[:, hs], in_=lg[:, hs],
                                     scale=-1.0,
                                     func=mybir.ActivationFunctionType.Exp)
                nc.tensor.matmul(rlb_ps[:, hs], oner, rl16[:, hs],
                                 start=True, stop=True)
                nc.vector.tensor_copy(out=rlb[:, hs], in_=rlb_ps[:, hs])
                nc.vector.tensor_tensor(out=outsp[:, hs],
                                        in0=ot_ps[s][:, hs],
                                        in1=rlb[:, hs], op=A.mult)
                nc.sync.dma_start(
                    out=outT[:, s * SPAN + i * half:s * SPAN + (i + 1) * half],
                    in_=outsp[:, hs])

        # ---- pipelined schedule ----
        # sweep A: spans 0,1 interleaved with projections as kT/qT arrive
        proj_quarter(0)
        unit(0, 0); unit(1, 0)
        proj_quarter(1)
        unit(2, 0); unit(3, 0)
        unit(0, 1); unit(1, 1); unit(2, 1); unit(3, 1)
        proj_quarter(2)
        unit(4, 0); unit(4, 1); unit(5, 0); unit(5, 1)
        proj_quarter(3)
        unit(6, 0); unit(6, 1)
        unit(7, 0)
        finalize(0)
        pf0 = unit_front(0, 2)
        unit(7, 1)
        finalize(1)
        pf1 = unit_front(1, 2)
        # sweep B: spans 2,3 (kT/qT/V all resident now)
        unit_back(0, 2, pf0)
        unit(0, 3)
        unit_back(1, 2, pf1)
        unit(1, 3)
        for p in range(2, NPAIR - 1):
            unit(p, 2)
            unit(p, 3)
        unit(NPAIR - 1, 2)
        finalize(2)
        unit(NPAIR - 1, 3)
        finalize(3)

    return nc


_CACHED = {}


def _get_nc(key="v2"):
    if key not in _CACHED:
        nc = build_nc()
        _split_excess_waits(nc)
        _CACHED[key] = nc
    return _CACHED[key]


def _make_in_maps(x, Wq, bq, Wk, bk, Wv, bv):
    def rnd16(a):
        return np.ascontiguousarray(np.asarray(a, np.float32), np.float16)

    xTm = rnd16(np.transpose(np.asarray(x, np.float32), (0, 2, 1)))

    def warr(w):
        w = np.asarray(w, np.float32).reshape(EC, 128, D)
        return rnd16(w.transpose(1, 0, 2).reshape(128, EC * D))

    Wq, Wk, Wv = warr(Wq), warr(Wk), warr(Wv)
    s_ = np.float32(D) ** np.float32(-0.25)
    bqc = np.ascontiguousarray(np.asarray(bq, np.float32) * s_)
    bkc = np.ascontiguousarray(np.asarray(bk, np.float32) * s_)
    bvc = np.ascontiguousarray(np.asarray(bv, np.float32))
    import ml_dtypes

    ident = np.eye(128, dtype=np.float32)
    onesv = np.ones((128,), np.float16)
    onesb = np.ones((128,), ml_dtypes.bfloat16)
    return [
        {"xT": np.ascontiguousarray(xTm[b]), "Wq": Wq, "Wk": Wk, "Wv": Wv,
         "bqc": bqc, "bkc": bkc, "bvc": bvc, "ident": ident,
         "onec": onesb, "oner": onesv}
        for b in range(B)
    ]


def kernel(x, Wq, bq, Wk, bk, Wv, bv, _trace=False, _mm_dt=None):
    from concourse.bass_utils import run_bass_kernel_spmd

    nc = _get_nc()
    in_maps = _make_in_maps(x, Wq, bq, Wk, bk, Wv, bv)
    res = run_bass_kernel_spmd(nc, in_maps, core_ids=list(range(B)),
                               trace=_trace)
    out = np.stack([np.ascontiguousarray(res.results[b]["outT"].T)
                    for b in range(B)])
    kernel._last_result = res
    return out


# revision 29
# speedup vs baseline: 1.0028x; 1.0028x over previous
"""Single-head attention (B=8, T=2048, E=1024, D=128) on 8 Trainium2 NeuronCores.

Strategy (data-parallel over batch, one batch element per core):
  host: pre-transpose x -> xT[b] = x[b].T (E on rows), cast to fp16;
        pre-scale q/k biases by D**-0.25.
  device, per core (flash-style, pipelined over T-quarters):
    - x streams in [128,1024] chunks on the two HW DGE queues (sync +
      scalar engines), weights interleaved so each lands just before
      first use; only the first ~8 queue slots execute eagerly, the rest
      are paced by the tile clock, so the order here matters a lot
    - 12 junk warm-up matmuls on the first weight chunk keep the PE busy
      through the DMA lead-in so the HAM clock-gate opens (2.4 GHz)
      before real work -- worth ~24us(!) on a ~90us kernel
    - per quarter h: kT/qT (fp16, scaled by D**-0.25) and vT via PE
      (fp16 matmuls, full rate); V natural [k, D] bf16 via PE transposes
    - score/AV "units" interleave with projections as soon as their kT
      block + qT span exist (sweep A: spans 0,1; sweep B: spans 2,3 --
      so 2 persistent ot banks + 4 score banks + 2 proj banks = 8):
        unit(kb-pair, s): 2 score MMs into a 2-bank PSUM pair,
        ONE exp over [128,1024] PSUM->SBUF bf16 (halving ACT overhead),
        2 AV MMs accumulating OT[d, q] into the span's persistent bank,
        softmax denominator accumulated on DVE in bf16 (2x mode; this
        replaces v1's 64 ones-matmuls, ~17us of PE time)
    - span finalize: l = ones[128,1].T @ acc (M=1 matmul); 1/l =
      exp(-ln(l)) on ACT (this walrus rejects custom-DVE reciprocal and
      plain DVE reciprocal is ~8 cyc/elem); broadcast across partitions
      via a K=1 matmul; OT * (1/l) on DVE; DMA out via sync queue only
      (the scalar queue shares the ACT engine, which exp saturates)
    - store outT [D, T] f32; host transposes back to [T, D].
"""

import os
import sys

for _p in ("/opt/trn_rl_repo",):
    if _p not in sys.path and os.path.isdir(_p):
        sys.path.append(_p)

import numpy as np

import concourse.bass as bass
import concourse.tile as tile
from concourse import mybir
from concourse.masks import make_identity
from concourse.vector_clock import ScopedClock

B, T, E, D = 8, 2048, 1024, 128
EC = E // 128           # 8 E-chunks of 128 partitions
NSPAN = 4               # query spans of 512
SPAN = T // NSPAN       # 512
NKB = T // 128          # 16 key blocks
NPAIR = NKB // 2        # 8 key-block pairs
F32 = mybir.dt.float32
F32R = mybir.dt.float32r
BF16 = mybir.dt.bfloat16
F16 = mybir.dt.float16

_MAX_DRAIN_WAITS = 1


def _drain_and_barrier_split(self, tick_clock, wait_clock):
    # This walrus build rejects CTRL instructions carrying more than one sync
    # wait, so spread the kernel-tail drain's waits over single-wait NOPs.
    nc = self.nc
    collector = nc.sync.nop(nofuse=True, hint="drain_wait_collector")
    wait_clock.add_sem_waits(
        collector.ins, ScopedClock({None: tick_clock.global_clock})
    )
    si = collector.ins.sync_info
    waits = list(si.on_wait) if si and si.on_wait else []
    if len(waits) > _MAX_DRAIN_WAITS:
        si.on_wait = waits[:_MAX_DRAIN_WAITS]
        rest = waits[_MAX_DRAIN_WAITS:]
        while rest:
            chunk, rest = rest[:_MAX_DRAIN_WAITS], rest[_MAX_DRAIN_WAITS:]
            extra = nc.sync.nop(nofuse=True, hint="drain_wait_extra")
            if extra.ins.sync_info is None:
                extra.ins.sync_info = type(si)(on_wait=chunk, on_update=[])
            else:
                extra.ins.sync_info.on_wait = chunk

    nc.sync.drain()

    nc.all_engine_barrier()
    assert self.sems is not None
    popped = nc._tile_sem_poison_stack.pop()
    assert popped is self._sem_poison
    nc.clear_and_free_semaphores(list(self.sems.allocated().values()))
    nc.all_engine_barrier()


tile.TileContext._drain_and_barrier = _drain_and_barrier_split


def _split_excess_waits(nc):
    """Walrus in this env allows at most one sync wait per instruction;
    hoist extra waits onto same-engine NOPs placed just before."""
    import copy

    m = nc.m
    cnt = 0
    new_funcs = []
    for function in m.functions:
        new_function = copy.replace(function, blocks=[])
        new_function.set_allocations_from_list(function.allocations)
        for block in function.blocks:
            new_insts = []
            for inst in block.instructions:
                si = inst.sync_info
                waits = list(si.on_wait) if si and si.on_wait else []
                if len(waits) > 1:
                    for w in waits[:-1]:
                        nop = mybir.InstNoOp(name=f"I-swsplit-{cnt}",
                                             ins=[], outs=[])
                        cnt += 1
                        nop.engine = inst.engine
                        nop.sync_info = mybir.SyncInfo(on_wait=[w],
                                                       on_update=[])
                        new_insts.append(nop)
                    si.on_wait = [waits[-1]]
                new_insts.append(inst)
            new_function.blocks.append(
                copy.replace(block, instructions=new_insts))
        new_funcs.append(new_function)
    new_m = copy.replace(m, functions=[])
    for f in new_funcs:
        new_m.functions.append(f)
    nc.m = new_m
    return cnt


def build_nc():
    SCALE = float(np.float32(D) ** np.float32(-0.25))
    H4 = T // 4             # 512, quarter width
    A = mybir.AluOpType

    nc = bass.Bass()
    xT = nc.declare_dram_parameter("xT", [E, T], F16, isOutput=False)[:]
    Wq = nc.declare_dram_parameter("Wq", [128, EC * D], F16, isOutput=False)[:]
    Wk = nc.declare_dram_parameter("Wk", [128, EC * D], F16, isOutput=False)[:]
    Wv = nc.declare_dram_parameter("Wv", [128, EC * D], F16, isOutput=False)[:]
    bqc = nc.declare_dram_parameter("bqc", [D], F32, isOutput=False)[:]
    bkc = nc.declare_dram_parameter("bkc", [D], F32, isOutput=False)[:]
    bvc = nc.declare_dram_parameter("bvc", [D], F32, isOutput=False)[:]
    ident_d = nc.declare_dram_parameter("ident", [128, 128], F32,
                                        isOutput=False)[:]
    onec_d = nc.declare_dram_parameter("onec", [128], BF16, isOutput=False)[:]
    oner_d = nc.declare_dram_parameter("oner", [128], F16, isOutput=False)[:]
    outT = nc.declare_dram_parameter("outT", [D, T], F32, isOutput=True)[:]

    with tile.TileContext(nc) as tc, \
         tc.tile_pool(name="consts", bufs=1) as consts, \
         tc.tile_pool(name="xpool", bufs=1) as xpool, \
         tc.tile_pool(name="persist", bufs=1) as persist, \
         tc.tile_pool(name="vtq", bufs=2) as vtqp, \
         tc.tile_pool(name="ppool", bufs=6) as ppool, \
         tc.tile_pool(name="fin", bufs=2) as finp, \
         tc.tile_pool(name="psO", bufs=2, space="PSUM") as psO, \
         tc.tile_pool(name="psS", bufs=2, space="PSUM") as psS, \
         tc.tile_pool(name="psP", bufs=2, space="PSUM") as psP:

        # ---- constants / weights (spread across both DGE queues) ----
        wq_s = consts.tile([128, EC, D], F16, tag="wq")
        wk_s = consts.tile([128, EC, D], F16, tag="wk")
        wv_s = consts.tile([128, EC, D], F16, tag="wv")
        wk_r = Wk.rearrange("p (c d) -> p c d", d=D)
        bq_s = consts.tile([128, 1], F32, tag="bq")
        bk_s = consts.tile([128, 1], F32, tag="bk")
        bv_s = consts.tile([128, 1], F32, tag="bv")
        ident = consts.tile([128, 128], F32, tag="ident")
        onec = consts.tile([128, 1], BF16, tag="onec")
        oner = consts.tile([1, 128], F16, tag="oner")

        # ---- x: [128,1024] chunks (half-T per E-chunk), 2 HW DGE queues;
        # weights interleaved so each tensor lands just before first use ----
        xh = [[xpool.tile([128, 2 * H4], F16, tag=f"x{e}_{half}",
                          name=f"xh{e}_{half}")
               for half in range(2)] for e in range(EC)]

        def xdma(eng, e, half):
            eng.dma_start(out=xh[e][half],
                          in_=xT[e * 128:(e + 1) * 128,
                                 half * 2 * H4:(half + 1) * 2 * H4])

        # sync queue: wk chunks first so k-proj starts immediately
        nc.sync.dma_start(out=wk_s[:, 0:2, :], in_=wk_r[:, 0:2, :])
        xdma(nc.sync, 0, 0)
        xdma(nc.sync, 2, 0)
        nc.sync.dma_start(out=wk_s[:, 2:EC, :], in_=wk_r[:, 2:EC, :])
        xdma(nc.sync, 4, 0)
        xdma(nc.sync, 6, 0)
        for e in (0, 2, 4, 6):
            xdma(nc.sync, e, 1)
        # scalar queue: x odds + weights interleaved; tiny consts after
        # (each DMA costs ~0.6us of queue time regardless of size, so
        # front-loading 6 tiny consts would starve the k projection)
        xdma(nc.scalar, 1, 0)
        xdma(nc.scalar, 3, 0)
        nc.scalar.dma_start(out=wv_s, in_=Wv.rearrange("p (c d) -> p c d",
                                                       d=D))
        xdma(nc.scalar, 5, 0)
        xdma(nc.scalar, 7, 0)
        nc.scalar.dma_start(out=wq_s,
                            in_=Wq.rearrange("p (c d) -> p c d", d=D))
        for b_s, b_d in ((bq_s, bqc), (bk_s, bkc), (bv_s, bvc)):
            nc.scalar.dma_start(out=b_s, in_=b_d.unsqueeze(1))
        nc.scalar.dma_start(out=ident, in_=ident_d)
        for e in (1, 3, 5, 7):
            xdma(nc.scalar, e, 1)
        nc.scalar.dma_start(out=onec, in_=onec_d.unsqueeze(1))
        nc.scalar.dma_start(out=oner, in_=oner_d.unsqueeze(0))

        # ---- HAM warm-up: junk matmuls on the first-arrived weight chunk
        # bridge the DMA lead-in so the PE clock gate opens before real
        # work. Target a score-pool bank (not needed until ~25us) so the
        # dummies never block the projection PSUM ring.
        n_dummy = int(os.environ.get("DUMMY_MM", "20"))
        if n_dummy:
            dmy_ps = psS.tile([128, 2 * SPAN], F32, tag="st",
                              name="dummy_ps")
            wk01 = wk_s.rearrange("p c d -> p (c d)")[:, 0:2 * D]
            for _ in range(n_dummy):
                nc.tensor.matmul(dmy_ps[:, 0:256], wk_s[:, 0, :], wk01,
                                 start=True, stop=True)

        def xq(e, h):
            # [128, 512] view of quarter h of E-chunk e
            return xh[e][h // 2][:, (h % 2) * H4:(h % 2 + 1) * H4]

        kT_s = persist.tile([128, T], F16, tag="kT")
        qT_s = persist.tile([128, T], F16, tag="qT")
        V_s = persist.tile([128, NKB, D], BF16, tag="V")
        acc = [persist.tile([128, SPAN], BF16, tag=f"acc{s}", name=f"acc{s}")
               for s in range(NSPAN)]

        ot_ps = [None] * NSPAN      # span -> persistent PSUM bank
        done_in_span = [0] * NSPAN  # kb-pairs accumulated so far

        def evict_scaled(dst, src, bias, on_act):
            # dst = src * SCALE + bias (bias pre-scaled on host)
            if on_act:
                nc.scalar.activation(out=dst, in_=src, scale=SCALE,
                                     bias=bias,
                                     func=mybir.ActivationFunctionType
                                     .Identity)
            else:
                nc.vector.tensor_scalar(out=dst, in0=src,
                                        scalar1=SCALE, scalar2=bias,
                                        op0=A.mult, op1=A.add)

        EORD = (1, 0, 3, 2, 5, 7, 4, 6)  # measured chunk arrival order

        def qproj(h):
            hsl = slice(h * H4, (h + 1) * H4)
            q_ps = psP.tile([128, H4], F32, tag="pj", name=f"q_ps{h}")
            for n, e in enumerate(EORD):
                nc.tensor.matmul(q_ps, wq_s[:, e, :], xq(e, h),
                                 start=(n == 0), stop=(n == EC - 1))
            evict_scaled(qT_s[:, hsl], q_ps, bq_s, on_act=False)

        def proj_quarter(h, do_q=True):
            hsl = slice(h * H4, (h + 1) * H4)
            k_ps = psP.tile([128, H4], F32, tag="pj", name=f"k_ps{h}")
            for n, e in enumerate(EORD):
                nc.tensor.matmul(k_ps, wk_s[:, e, :], xq(e, h),
                                 start=(n == 0), stop=(n == EC - 1))
            evict_scaled(kT_s[:, hsl], k_ps, bk_s, on_act=False)
            v_ps = psP.tile([128, H4], F32, tag="pj", name=f"v_ps{h}")
            for n, e in enumerate(EORD):
                nc.tensor.matmul(v_ps, wv_s[:, e, :], xq(e, h),
                                 start=(n == 0), stop=(n == EC - 1))
            vtq = vtqp.tile([128, H4], F32, tag="vtq", name=f"vtq{h}")
            nc.vector.tensor_scalar(out=vtq, in0=v_ps,
                                    scalar1=bv_s, scalar2=None, op0=A.add)
            if do_q:
                qproj(h)
            # V natural layout [k, D] for the 4 new key blocks
            for j in range(4):
                kb = 4 * h + j
                vt_ps = psP.tile([128, 128], F32, tag="pj", name=f"vt{kb}")
                nc.tensor.transpose(vt_ps, vtq[:, j * 128:(j + 1) * 128],
                                    ident)
                nc.vector.tensor_copy(out=V_s[:, kb, :], in_=vt_ps)

        def unit_front(p, s):
            # scores + exp for one kb-pair (blocks 2p, 2p+1) against span s
            ssl = slice(s * SPAN, (s + 1) * SPAN)
            st2 = psS.tile([128, 2 * SPAN], F32, tag="st", name=f"st{p}_{s}")
            for i in range(2):
                kb = 2 * p + i
                nc.tensor.matmul(st2[:, i * SPAN:(i + 1) * SPAN],
                                 kT_s[:, kb * 128:(kb + 1) * 128],
                                 qT_s[:, ssl], start=True, stop=True)
            p2 = ppool.tile([128, 2 * SPAN], BF16, tag="p2", name=f"p{p}_{s}")
            nc.scalar.activation(out=p2, in_=st2,
                                 func=mybir.ActivationFunctionType.Exp)
            return p2

        def unit_back(p, s, p2):
            # AV accumulation + denominator accumulation
            if ot_ps[s] is None:
                ot_ps[s] = psO.tile([128, SPAN], F32, tag="ot",
                                    name=f"ot{s}")
            first = done_in_span[s] == 0
            for i in range(2):
                kb = 2 * p + i
                nc.tensor.matmul(ot_ps[s], V_s[:, kb, :],
                                 p2[:, i * SPAN:(i + 1) * SPAN],
                                 start=(first and i == 0),
                                 stop=(done_in_span[s] == NPAIR - 1
                                       and i == 1))
            # denominator partial sums (bf16, values <= ~2.4e3)
            if first:
                nc.vector.tensor_tensor(out=acc[s], in0=p2[:, 0:SPAN],
                                        in1=p2[:, SPAN:], op=A.add)
            else:
                nc.vector.tensor_tensor(out=acc[s], in0=acc[s],
                                        in1=p2[:, 0:SPAN], op=A.add)
                nc.vector.tensor_tensor(out=acc[s], in0=acc[s],
                                        in1=p2[:, SPAN:], op=A.add)
            done_in_span[s] += 1

        def unit(p, s):
            unit_back(p, s, unit_front(p, s))

        def finalize(s):
            # 1/l = exp(-ln(l)) on ACT -- walrus here rejects the custom-DVE
            # fast-reciprocal, and plain DVE reciprocal is ~8 cyc/elem.
            # Chain is column-half pipelined to shorten the exposed tail.
            lr_ps = psP.tile([1, SPAN], F32, tag="pj", name=f"lr{s}")
            nc.tensor.matmul(lr_ps, onec, acc[s], start=True, stop=True)
            lg = finp.tile([1, SPAN], F32, tag="lg", name=f"lg{s}")
            rl16 = finp.tile([1, SPAN], F16, tag="rl16", name=f"rl16{s}")
            rlb_ps = psP.tile([128, SPAN], F32, tag="pj", name=f"rlb{s}")
            rlb = finp.tile([128, SPAN], F32, tag="rlb", name=f"rlbs{s}")
            outsp = finp.tile([128, SPAN], F32, tag="out", name=f"out{s}")
            half = SPAN // 2
            for i in range(2):
                hs = slice(i * half, (i + 1) * half)
                nc.scalar.activation(out=lg[:, hs], in_=lr_ps[:, hs],
                                     func=mybir.ActivationFunctionType.Ln)
                nc.scalar.activation(out=rl16[:, hs], in_=lg[:, hs],
                                     scale=-1.0,
                                     func=mybir.ActivationFunctionType.Exp)
                nc.tensor.matmul(rlb_ps[:, hs], oner, rl16[:, hs],
                                 start=True, stop=True)
                nc.vector.tensor_copy(out=rlb[:, hs], in_=rlb_ps[:, hs])
                nc.vector.tensor_tensor(out=outsp[:, hs],
                                        in0=ot_ps[s][:, hs],
                                        in1=rlb[:, hs], op=A.mult)
                nc.sync.dma_start(
                    out=outT[:, s * SPAN + i * half:s * SPAN + (i + 1) * half],
                    in_=outsp[:, hs])

        # ---- pipelined schedule ----
        # sweep A: spans 0,1 interleaved with projections as kT/qT arrive
        proj_quarter(0)
        unit(0, 0); unit(1, 0)
        proj_quarter(1)
        unit(2, 0); unit(3, 0)
        unit(0, 1); unit(1, 1); unit(2, 1); unit(3, 1)
        proj_quarter(2)
        unit(4, 0); unit(4, 1); unit(5, 0); unit(5, 1)
        proj_quarter(3)
        unit(6, 0); unit(6, 1)
        unit(7, 0)
        finalize(0)
        pf0 = unit_front(0, 2)
        unit(7, 1)
        finalize(1)
        pf1 = unit_front(1, 2)
        pf03 = unit_front(0, 3)
        # sweep B: spans 2,3 (kT/qT/V all resident now)
        unit_back(0, 2, pf0)
        unit_back(0, 3, pf03)
        unit_back(1, 2, pf1)
        unit(1, 3)
        for p in range(2, NPAIR - 1):
            unit(p, 2)
            unit(p, 3)
        unit(NPAIR - 1, 2)
        finalize(2)
        unit(NPAIR - 1, 3)
        finalize(3)

    return nc


_CACHED = {}


def _get_nc(key="v2"):
    if key not in _CACHED:
        nc = build_nc()
        _split_excess_waits(nc)
        _CACHED[key] = nc
    return _CACHED[key]


def _make_in_maps(x, Wq, bq, Wk, bk, Wv, bv):
    def rnd16(a):
        return np.ascontiguousarray(np.asarray(a, np.float32), np.float16)

    xTm = rnd16(np.transpose(np.asarray(x, np.float32), (0, 2, 1)))

    def warr(w):
        w = np.asarray(w, np.float32).reshape(EC, 128, D)
        return rnd16(w.transpose(1, 0, 2).reshape(128, EC * D))

    Wq, Wk, Wv = warr(Wq), warr(Wk), warr(Wv)
    s_ = np.float32(D) ** np.float32(-0.25)
    bqc = np.ascontiguousarray(np.asarray(bq, np.float32) * s_)
    bkc = np.ascontiguousarray(np.asarray(bk, np.float32) * s_)
    bvc = np.ascontiguousarray(np.asarray(bv, np.float32))
    import ml_dtypes

    ident = np.eye(128, dtype=np.float32)
    onesv = np.ones((128,), np.float16)
    onesb = np.ones((128,), ml_dtypes.bfloat16)
    return [
        {"xT": np.ascontiguousarray(xTm[b]), "Wq": Wq, "Wk": Wk, "Wv": Wv,
         "bqc": bqc, "bkc": bkc, "bvc": bvc, "ident": ident,
         "onec": onesb, "oner": onesv}
        for b in range(B)
    ]


def kernel(x, Wq, bq, Wk, bk, Wv, bv, _trace=False, _mm_dt=None):
    from concourse.bass_utils import run_bass_kernel_spmd

    nc = _get_nc()
    in_maps = _make_in_maps(x, Wq, bq, Wk, bk, Wv, bv)
    res = run_bass_kernel_spmd(nc, in_maps, core_ids=list(range(B)),
                               trace=_trace)
    out = np.stack([np.ascontiguousarray(res.results[b]["outT"].T)
                    for b in range(B)])
    kernel._last_result = res
    return out
